# revision 4
# baseline (speedup 1.0000x reference)
"""CSWin block kernel — 8-way batch-data-parallel on Trainium2 NeuronCores.

Sharding: pure data parallel — batch B=8 split one element per NeuronCore;
all params (<20KB) replicated; CSWin windows never cross batch elements so
shards are fully independent (no collectives).

Implementation note: executed as four sequential pmapped stages (qkv,
branch-0 attention, branch-1 attention, proj/LN/residual). The monolithic
graph trips an internal neuronxcc tensorizer assert (PComputeCutting); the
staged graphs compile cleanly and intermediates stay sharded on-device.

Self-contained: shapes hardcoded for x:(8,2048,2048) f32.
"""

import numpy as np
import jax
import jax.numpy as jnp

B, H, W = 8, 2048, 2048
C = 16
HALF = 8
SPLIT = 8
EPS = 1e-5
Hg, Wg = H // 4, W // 4
N = Hg * Wg


def _attn(qw, kw, vw, taps, bias):
    """Windowed softmax attention + LePE. qw/kw/vw: (nWin, 8, HALF)."""
    scale = HALF ** -0.5
    s = ((qw * scale)[:, :, None, :] * kw[:, None, :, :]).sum(-1)
    s = s - s.max(-1, keepdims=True)
    e = jnp.exp(s)
    p = e / e.sum(-1, keepdims=True)
    out = (p[:, :, :, None] * vw[:, None, :, :]).sum(2)
    vpad = jnp.pad(vw, ((0, 0), (1, 1), (0, 0)))
    lepe = (vpad[:, 0:SPLIT, :] * taps[:, 0][None, None, :]
            + vw * taps[:, 1][None, None, :]
            + vpad[:, 2:SPLIT + 2, :] * taps[:, 2][None, None, :]
            + bias[None, None, :])
    return out + lepe


def _stage_qkv(x, qkv_w):
    return x.reshape(N, C) @ qkv_w                      # (N, 48)


def _stage_b0(qkv, t0, b0):
    q0 = qkv[:, 0:HALF].reshape(-1, SPLIT, HALF)
    k0 = qkv[:, 16:16 + HALF].reshape(-1, SPLIT, HALF)
    v0 = qkv[:, 32:32 + HALF].reshape(-1, SPLIT, HALF)
    return _attn(q0, k0, v0, t0, b0).reshape(N, HALF)


def _stage_b1(qkv, t1, b1):
    def to_win(t):
        t = t.reshape(Hg // SPLIT, SPLIT, Wg, HALF)
        return t.transpose(0, 2, 1, 3).reshape(-1, SPLIT, HALF)

    o = _attn(to_win(qkv[:, HALF:16]), to_win(qkv[:, 16 + HALF:32]),
              to_win(qkv[:, 32 + HALF:48]), t1, b1)
    o = o.reshape(Hg // SPLIT, Wg, SPLIT, HALF).transpose(0, 2, 1, 3)
    return o.reshape(N, HALF)


def _stage_tail(x, x1, x2, proj_w, proj_b, ln_w, ln_b):
    tok = x.reshape(N, C)
    att = jnp.concatenate([x1, x2], axis=-1) @ proj_w + proj_b
    mu = att.mean(-1, keepdims=True)
    var = ((att - mu) ** 2).mean(-1, keepdims=True)
    normed = (att - mu) * jax.lax.rsqrt(var + EPS) * ln_w + ln_b
    return (tok + normed).T.reshape(H, W)


_STAGES = None


def _build(devs):
    pm = lambda f, n: jax.pmap(f, in_axes=(0,) * n + (None,) * 9, devices=devs)
    s1 = jax.pmap(_stage_qkv, in_axes=(0, None), devices=devs)
    s2 = jax.pmap(_stage_b0, in_axes=(0, None, None), devices=devs)
    s3 = jax.pmap(_stage_b1, in_axes=(0, None, None), devices=devs)
    s4 = jax.pmap(_stage_tail, in_axes=(0, 0, 0, None, None, None, None),
                  devices=devs)
    return s1, s2, s3, s4


def kernel(x, qkv_w, conv_w0, conv_b0, conv_w1, conv_b1, proj_w, proj_b,
           ln_w, ln_b):
    global _STAGES
    t0 = np.ascontiguousarray(np.asarray(conv_w0, np.float32)[:, 0, 1, :])
    t1 = np.ascontiguousarray(np.asarray(conv_w1, np.float32)[:, 0, :, 1])
    x = np.asarray(x, np.float32)
    qkv_w = np.asarray(qkv_w, np.float32)
    conv_b0 = np.asarray(conv_b0, np.float32)
    conv_b1 = np.asarray(conv_b1, np.float32)
    proj_w = np.asarray(proj_w, np.float32)
    proj_b = np.asarray(proj_b, np.float32)
    ln_w = np.asarray(ln_w, np.float32)
    ln_b = np.asarray(ln_b, np.float32)

    devs = [d for d in jax.devices() if d.platform != "cpu"][:B]
    if len(devs) == B:
        if _STAGES is None:
            _STAGES = _build(devs)
        s1, s2, s3, s4 = _STAGES
        qkv = s1(x, qkv_w)
        x1 = s2(qkv, t0, conv_b0)
        x2 = s3(qkv, t1, conv_b1)
        out = s4(x, x1, x2, proj_w, proj_b, ln_w, ln_b)
        return np.asarray(jax.device_get(out), np.float32)

    # Fallback: single-device path (correctness preserved).
    def one(xb):
        qkv = _stage_qkv(xb, qkv_w)
        return _stage_tail(xb, _stage_b0(qkv, t0, conv_b0),
                           _stage_b1(qkv, t1, conv_b1),
                           proj_w, proj_b, ln_w, ln_b)

    out = jax.jit(jax.vmap(one))(x)
    return np.asarray(jax.device_get(out), np.float32)


# revision 6
# speedup vs baseline: 76.9148x; 76.9148x over previous
"""CSWin block kernel — 8-way batch-data-parallel on Trainium2 NeuronCores.

Sharding: pure data parallel — batch B=8 split one element per NeuronCore;
all params (<20KB) replicated; CSWin windows never cross batch elements so
shards are fully independent (no collectives).

Implementation note: executed as four sequential pmapped stages (qkv,
branch-0 attention, branch-1 attention, proj/LN/residual). The monolithic
graph trips an internal neuronxcc tensorizer assert (PComputeCutting); the
staged graphs compile cleanly and intermediates stay sharded on-device.

Self-contained: shapes hardcoded for x:(8,2048,2048) f32.
"""

import numpy as np
import jax
import jax.numpy as jnp

B, H, W = 8, 2048, 2048
C = 16
HALF = 8
SPLIT = 8
EPS = 1e-5
Hg, Wg = H // 4, W // 4
N = Hg * Wg


def _attn(qw, kw, vw, taps, bias):
    """Windowed softmax attention + LePE. qw/kw/vw: (nWin, 8, HALF)."""
    scale = HALF ** -0.5
    s = ((qw * scale)[:, :, None, :] * kw[:, None, :, :]).sum(-1)
    s = s - s.max(-1, keepdims=True)
    e = jnp.exp(s)
    p = e / e.sum(-1, keepdims=True)
    out = (p[:, :, :, None] * vw[:, None, :, :]).sum(2)
    vpad = jnp.pad(vw, ((0, 0), (1, 1), (0, 0)))
    lepe = (vpad[:, 0:SPLIT, :] * taps[:, 0][None, None, :]
            + vw * taps[:, 1][None, None, :]
            + vpad[:, 2:SPLIT + 2, :] * taps[:, 2][None, None, :]
            + bias[None, None, :])
    return out + lepe


def _stage_qkv(x, qkv_w):
    return x.reshape(N, C) @ qkv_w                      # (N, 48)


def _stage_b0(qkv, t0, b0):
    q0 = qkv[:, 0:HALF].reshape(-1, SPLIT, HALF)
    k0 = qkv[:, 16:16 + HALF].reshape(-1, SPLIT, HALF)
    v0 = qkv[:, 32:32 + HALF].reshape(-1, SPLIT, HALF)
    return _attn(q0, k0, v0, t0, b0).reshape(N, HALF)


def _stage_b1(qkv, t1, b1):
    def to_win(t):
        t = t.reshape(Hg // SPLIT, SPLIT, Wg, HALF)
        return t.transpose(0, 2, 1, 3).reshape(-1, SPLIT, HALF)

    o = _attn(to_win(qkv[:, HALF:16]), to_win(qkv[:, 16 + HALF:32]),
              to_win(qkv[:, 32 + HALF:48]), t1, b1)
    o = o.reshape(Hg // SPLIT, Wg, SPLIT, HALF).transpose(0, 2, 1, 3)
    return o.reshape(N, HALF)


def _stage_tail(x, x1, x2, proj_w, proj_b, ln_w, ln_b):
    tok = x.reshape(N, C)
    att = jnp.concatenate([x1, x2], axis=-1) @ proj_w + proj_b
    mu = att.mean(-1, keepdims=True)
    var = ((att - mu) ** 2).mean(-1, keepdims=True)
    normed = (att - mu) * jax.lax.rsqrt(var + EPS) * ln_w + ln_b
    return (tok + normed).T.reshape(H, W)


def _stage_rest(x, qkv, t0, b0, t1, b1, proj_w, proj_b, ln_w, ln_b):
    return _stage_tail(x, _stage_b0(qkv, t0, b0), _stage_b1(qkv, t1, b1),
                       proj_w, proj_b, ln_w, ln_b)


_STAGES = None


def _build(devs):
    # Two dispatches: the qkv matmul must stay in its own program — fusing it
    # with the attention stages reproduces the neuronxcc PGTiling assert.
    s1 = jax.pmap(_stage_qkv, in_axes=(0, None), devices=devs)
    s2 = jax.pmap(_stage_rest, in_axes=(0, 0) + (None,) * 8, devices=devs)
    return s1, s2


def kernel(x, qkv_w, conv_w0, conv_b0, conv_w1, conv_b1, proj_w, proj_b,
           ln_w, ln_b):
    global _STAGES
    t0 = np.ascontiguousarray(np.asarray(conv_w0, np.float32)[:, 0, 1, :])
    t1 = np.ascontiguousarray(np.asarray(conv_w1, np.float32)[:, 0, :, 1])
    x = np.asarray(x, np.float32)
    qkv_w = np.asarray(qkv_w, np.float32)
    conv_b0 = np.asarray(conv_b0, np.float32)
    conv_b1 = np.asarray(conv_b1, np.float32)
    proj_w = np.asarray(proj_w, np.float32)
    proj_b = np.asarray(proj_b, np.float32)
    ln_w = np.asarray(ln_w, np.float32)
    ln_b = np.asarray(ln_b, np.float32)

    devs = [d for d in jax.devices() if d.platform != "cpu"][:B]
    if len(devs) == B:
        if _STAGES is None:
            _STAGES = _build(devs)
        s1, s2 = _STAGES
        xs = jax.device_put_sharded(list(x), devs)
        qkv = s1(xs, qkv_w)
        out = s2(xs, qkv, t0, conv_b0, t1, conv_b1,
                 proj_w, proj_b, ln_w, ln_b)
        return np.asarray(jax.device_get(out), np.float32)

    # Fallback: single-device path (correctness preserved).
    def one(xb):
        qkv = _stage_qkv(xb, qkv_w)
        return _stage_tail(xb, _stage_b0(qkv, t0, conv_b0),
                           _stage_b1(qkv, t1, conv_b1),
                           proj_w, proj_b, ln_w, ln_b)

    out = jax.jit(jax.vmap(one))(x)
    return np.asarray(jax.device_get(out), np.float32)


# revision 7
# speedup vs baseline: 382.9667x; 4.9791x over previous
"""CSWin block kernel — 8-way batch-data-parallel on Trainium2 NeuronCores.

Sharding: pure data parallel — batch B=8 split one element per NeuronCore;
all params (<20KB) replicated; CSWin windows never cross batch elements so
shards are fully independent (no collectives).

Implementation note: executed as four sequential pmapped stages (qkv,
branch-0 attention, branch-1 attention, proj/LN/residual). The monolithic
graph trips an internal neuronxcc tensorizer assert (PComputeCutting); the
staged graphs compile cleanly and intermediates stay sharded on-device.

Self-contained: shapes hardcoded for x:(8,2048,2048) f32.
"""

import numpy as np
import jax
import jax.numpy as jnp

B, H, W = 8, 2048, 2048
C = 16
HALF = 8
SPLIT = 8
EPS = 1e-5
Hg, Wg = H // 4, W // 4
N = Hg * Wg


def _attn(qw, kw, vw, taps, bias):
    """Windowed softmax attention + LePE. qw/kw/vw: (nWin, 8, HALF)."""
    scale = HALF ** -0.5
    s = ((qw * scale)[:, :, None, :] * kw[:, None, :, :]).sum(-1)
    s = s - s.max(-1, keepdims=True)
    e = jnp.exp(s)
    p = e / e.sum(-1, keepdims=True)
    out = (p[:, :, :, None] * vw[:, None, :, :]).sum(2)
    vpad = jnp.pad(vw, ((0, 0), (1, 1), (0, 0)))
    lepe = (vpad[:, 0:SPLIT, :] * taps[:, 0][None, None, :]
            + vw * taps[:, 1][None, None, :]
            + vpad[:, 2:SPLIT + 2, :] * taps[:, 2][None, None, :]
            + bias[None, None, :])
    return out + lepe


def _stage_qkv(x, qkv_w):
    return x.reshape(N, C) @ qkv_w                      # (N, 48)


def _stage_b0(qkv, t0, b0):
    q0 = qkv[:, 0:HALF].reshape(-1, SPLIT, HALF)
    k0 = qkv[:, 16:16 + HALF].reshape(-1, SPLIT, HALF)
    v0 = qkv[:, 32:32 + HALF].reshape(-1, SPLIT, HALF)
    return _attn(q0, k0, v0, t0, b0).reshape(N, HALF)


def _stage_b1(qkv, t1, b1):
    # Vertical 8x1 windows: window = (row-group a, :, column j). Attention
    # runs over axis 1 of (64, 8, 512, h) directly — no transposes; the
    # direct reshape back to (N, h) is already grid-row-major token order.
    def w(i0):
        return qkv[:, i0 + HALF:i0 + 16].reshape(Hg // SPLIT, SPLIT, Wg, HALF)

    q1, k1, v1 = w(0), w(16), w(32)
    scale = HALF ** -0.5
    s = ((q1 * scale)[:, :, None, :, :] * k1[:, None, :, :, :]).sum(-1)
    s = s - s.max(2, keepdims=True)
    e = jnp.exp(s)
    p = e / e.sum(2, keepdims=True)
    out = (p[:, :, :, :, None] * v1[:, None, :, :, :]).sum(2)
    vpad = jnp.pad(v1, ((0, 0), (1, 1), (0, 0), (0, 0)))
    lepe = (vpad[:, 0:SPLIT] * t1[:, 0][None, None, None, :]
            + v1 * t1[:, 1][None, None, None, :]
            + vpad[:, 2:SPLIT + 2] * t1[:, 2][None, None, None, :]
            + b1[None, None, None, :])
    return (out + lepe).reshape(N, HALF)


def _stage_tail(x, x1, x2, proj_w, proj_b, ln_w, ln_b):
    tok = x.reshape(N, C)
    att = jnp.concatenate([x1, x2], axis=-1) @ proj_w + proj_b
    mu = att.mean(-1, keepdims=True)
    var = ((att - mu) ** 2).mean(-1, keepdims=True)
    normed = (att - mu) * jax.lax.rsqrt(var + EPS) * ln_w + ln_b
    return (tok + normed).T.reshape(H, W)


def _stage_rest(x, qkv, t0, b0, t1, b1, proj_w, proj_b, ln_w, ln_b):
    return _stage_tail(x, _stage_b0(qkv, t0, b0), _stage_b1(qkv, t1, b1),
                       proj_w, proj_b, ln_w, ln_b)


_STAGES = None


def _build(devs):
    # Two dispatches: the qkv matmul must stay in its own program — fusing it
    # with the attention stages reproduces the neuronxcc PGTiling assert.
    s1 = jax.pmap(_stage_qkv, in_axes=(0, None), devices=devs)
    s2 = jax.pmap(_stage_rest, in_axes=(0, 0) + (None,) * 8, devices=devs)
    return s1, s2


def kernel(x, qkv_w, conv_w0, conv_b0, conv_w1, conv_b1, proj_w, proj_b,
           ln_w, ln_b):
    global _STAGES
    t0 = np.ascontiguousarray(np.asarray(conv_w0, np.float32)[:, 0, 1, :])
    t1 = np.ascontiguousarray(np.asarray(conv_w1, np.float32)[:, 0, :, 1])
    x = np.asarray(x, np.float32)
    qkv_w = np.asarray(qkv_w, np.float32)
    conv_b0 = np.asarray(conv_b0, np.float32)
    conv_b1 = np.asarray(conv_b1, np.float32)
    proj_w = np.asarray(proj_w, np.float32)
    proj_b = np.asarray(proj_b, np.float32)
    ln_w = np.asarray(ln_w, np.float32)
    ln_b = np.asarray(ln_b, np.float32)

    devs = [d for d in jax.devices() if d.platform != "cpu"][:B]
    if len(devs) == B:
        if _STAGES is None:
            _STAGES = _build(devs)
        s1, s2 = _STAGES
        xs = jax.device_put_sharded(list(x), devs)
        qkv = s1(xs, qkv_w)
        out = s2(xs, qkv, t0, conv_b0, t1, conv_b1,
                 proj_w, proj_b, ln_w, ln_b)
        return np.asarray(jax.device_get(out), np.float32)

    # Fallback: single-device path (correctness preserved).
    def one(xb):
        qkv = _stage_qkv(xb, qkv_w)
        return _stage_tail(xb, _stage_b0(qkv, t0, conv_b0),
                           _stage_b1(qkv, t1, conv_b1),
                           proj_w, proj_b, ln_w, ln_b)

    out = jax.jit(jax.vmap(one))(x)
    return np.asarray(jax.device_get(out), np.float32)
